# revision 3
# baseline (speedup 1.0000x reference)
"""Trainium2 Bass kernel for nn_Add_31318901522623 (probabilistic ripple-carry adder).

Math: for k=2 digit distributions (p = P(bit=1)), the reference's einsum chain
collapses to a scalar linear recurrence per batch element.  In the "sign domain"
s = 1 - 2*P(1):
    s_xor  = s_a * s_b                      (XOR of independent bits)
    s_maj  = (s_a + s_b + s_c - s_a*s_b*s_c)/2   (majority = carry-out)
With sp' = (0.5 - p), sq' = (0.5 - q):
    w  = sp' * sq'        ( = s_a*s_b / 4 )
    u  = 0.5 - 2*w        ( = P(a xor b = 1), the carry-propagate prob )
    t  = sp' + sq'        ( = (s_a + s_b)/2 )
    sr_{i+1} = u_i * sr_i + t_i             (sr = sign of carry, sr_0 = +1)
    res1_i = 0.5 - 2 * w_i * sr_i           (sr_i = carry-IN sign)
    res0_i = 0.5 + 2 * w_i * sr_i

The recurrence maps 1:1 onto the VectorEngine's tensor_tensor_scan
(state = data0*state + data1 along the free dim).  Layout is batch-major:
each SBUF partition holds R batch rows of 64 positions; rows are padded to 65
with a reset element (u=0, t=1) so one scan instruction chains all R rows and
re-initializes the carry to +1 at each row boundary.  The scan output shifted
right by one column is then exactly the per-position exclusive carry-in.

Sharding: pure data parallel, B=262144 split as 32768 rows per NeuronCore,
zero cross-core communication.
"""

import os
import sys

import numpy as np

for _p in ("/opt/trn_rl_repo", "/root/.axon_site/_ro/trn_rl_repo"):
    if _p not in sys.path and os.path.isdir(_p):
        sys.path.append(_p)

from concourse import bacc, bass, mybir, tile
from concourse.bass_utils import run_bass_kernel_spmd

N_CORES = 8
B = 262144
L = 64
K = 2
B_LOCAL = B // N_CORES  # 32768
P = 128                 # SBUF partitions
R = 32                  # batch rows per partition per tile
TILE_ROWS = P * R       # 4096
N_TILES = B_LOCAL // TILE_ROWS  # 8

F32 = mybir.dt.float32
ALU = mybir.AluOpType
ACT_COPY = mybir.ActivationFunctionType.Copy


def build_program(reps: int = 1) -> bass.Bass:
    nc = bacc.Bacc(
        "TRN2",
        target_bir_lowering=False,
        debug=False,
        enable_asserts=False,
        num_devices=N_CORES,
    )

    d_op1 = nc.dram_tensor("op1", [B_LOCAL, L, K], F32, kind="ExternalInput").ap()
    d_op2 = nc.dram_tensor("op2", [B_LOCAL, L, K], F32, kind="ExternalInput").ap()
    d_out = nc.dram_tensor("out", [B_LOCAL, L, K], F32, kind="ExternalOutput").ap()

    with tile.TileContext(nc) as tc:
        with (
            tc.tile_pool(name="io", bufs=2) as io_pool,
            tc.tile_pool(name="scr", bufs=2) as scr_pool,
        ):
            for t in range(N_TILES * reps):
                t = t % N_TILES
                rows = slice(t * TILE_ROWS, (t + 1) * TILE_ROWS)

                in1 = io_pool.tile([P, R * L * K], F32, tag="in1")
                in2 = io_pool.tile([P, R * L * K], F32, tag="in2")
                nc.sync.dma_start(
                    out=in1[:],
                    in_=d_op1[rows].rearrange("(p r) l k -> p (r l k)", p=P),
                )
                nc.sync.dma_start(
                    out=in2[:],
                    in_=d_op2[rows].rearrange("(p r) l k -> p (r l k)", p=P),
                )

                # sp' = 0.5 - p, sq' = 0.5 - q  (strided read of the
                # interleaved [l, k] rows; ScalarE so the VectorEngine only
                # runs the 6 ops on the critical path)
                spp = scr_pool.tile([P, R * L], F32, tag="spp")
                sqp = scr_pool.tile([P, R * L], F32, tag="sqp")
                spp3 = spp[:].rearrange("p (r c) -> p r c", c=L)
                sqp3 = sqp[:].rearrange("p (r c) -> p r c", c=L)
                in1v = in1[:].rearrange("p (r l k) -> p r l k", l=L, k=K)
                in2v = in2[:].rearrange("p (r l k) -> p r l k", l=L, k=K)
                nc.scalar.activation(
                    out=spp3, in_=in1v[:, :, :, 1], func=ACT_COPY, bias=0.5, scale=-1.0
                )
                nc.scalar.activation(
                    out=sqp3, in_=in2v[:, :, :, 1], func=ACT_COPY, bias=0.5, scale=-1.0
                )

                # scan operands, rows padded to 65 with the reset element
                u_ext = scr_pool.tile([P, R * (L + 1)], F32, tag="u_ext")
                t_ext = scr_pool.tile([P, R * (L + 1)], F32, tag="t_ext")
                u3 = u_ext[:].rearrange("p (r c) -> p r c", c=L + 1)
                t3 = t_ext[:].rearrange("p (r c) -> p r c", c=L + 1)
                nc.gpsimd.memset(u3[:, :, L], 0.0)
                nc.gpsimd.memset(t3[:, :, L], 1.0)

                # t = sp' + sq'  (must read spp before the in-place w below)
                nc.vector.tensor_tensor(
                    out=t3[:, :, 0:L], in0=spp3, in1=sqp3, op=ALU.add
                )
                # w = sp' * sq'  (in place over spp)
                nc.vector.tensor_tensor(out=spp3, in0=spp3, in1=sqp3, op=ALU.mult)
                # u = -2*w + 0.5
                nc.vector.tensor_scalar(
                    out=u3[:, :, 0:L],
                    in0=spp3,
                    scalar1=-2.0,
                    scalar2=0.5,
                    op0=ALU.mult,
                    op1=ALU.add,
                )

                # carry scan: sr[n] = u[n]*sr[n-1] + t[n], written at offset 1
                # so srx (offset 0) is the exclusive carry-in; column 0 and the
                # reset columns hold +1 (carry=0).
                sr = scr_pool.tile([P, 1 + R * (L + 1)], F32, tag="sr")
                nc.gpsimd.memset(sr[:, 0:1], 1.0)
                nc.vector.tensor_tensor_scan(
                    out=sr[:, 1 : 1 + R * (L + 1)],
                    data0=u_ext[:],
                    data1=t_ext[:],
                    initial=1.0,
                    op0=ALU.mult,
                    op1=ALU.add,
                )
                srx = sr[:, 0 : R * (L + 1)].rearrange("p (r c) -> p r c", c=L + 1)[
                    :, :, 0:L
                ]

                # z = w * sr_in (in place over spp, which holds w)
                nc.vector.tensor_tensor(out=spp3, in0=spp3, in1=srx, op=ALU.mult)

                out_t = io_pool.tile([P, R * L * K], F32, tag="out")
                o4 = out_t[:].rearrange("p (r l k) -> p r l k", l=L, k=K)
                # res1 = -2*z + 0.5 (DVE), res0 = 2*z + 0.5 (ScalarE)
                nc.vector.tensor_scalar(
                    out=o4[:, :, :, 1],
                    in0=spp3,
                    scalar1=-2.0,
                    scalar2=0.5,
                    op0=ALU.mult,
                    op1=ALU.add,
                )
                nc.scalar.activation(
                    out=o4[:, :, :, 0], in_=spp3, func=ACT_COPY, bias=0.5, scale=2.0
                )

                nc.gpsimd.dma_start(
                    out=d_out[rows].rearrange("(p r) l k -> p (r l k)", p=P),
                    in_=out_t[:],
                )

    nc.compile()
    return nc


_NC = None


def _get_nc():
    global _NC
    if _NC is None:
        _NC = build_program()
    return _NC


def kernel(op1: np.ndarray, op2: np.ndarray) -> np.ndarray:
    op1 = np.ascontiguousarray(op1, dtype=np.float32)
    op2 = np.ascontiguousarray(op2, dtype=np.float32)
    assert op1.shape == (B, L, K) and op2.shape == (B, L, K)

    nc = _get_nc()
    in_maps = [
        {
            "op1": op1[i * B_LOCAL : (i + 1) * B_LOCAL],
            "op2": op2[i * B_LOCAL : (i + 1) * B_LOCAL],
        }
        for i in range(N_CORES)
    ]
    res = run_bass_kernel_spmd(nc, in_maps, core_ids=list(range(N_CORES)))
    return np.concatenate([res.results[i]["out"] for i in range(N_CORES)], axis=0)


# revision 7
# speedup vs baseline: 2.1195x; 2.1195x over previous
"""Trainium2 Bass kernel for nn_Add_31318901522623 (probabilistic ripple-carry adder).

Math: for k=2 digit distributions (p = P(bit=1)), the reference's einsum chain
collapses to a scalar linear recurrence per batch element.  In the "sign domain"
s = 1 - 2*P(1):
    s_xor  = s_a * s_b                      (XOR of independent bits)
    s_maj  = (s_a + s_b + s_c - s_a*s_b*s_c)/2   (majority = carry-out)
With sp' = (0.5 - p), sq' = (0.5 - q):
    w  = sp' * sq'        ( = s_a*s_b / 4 )
    u  = 0.5 - 2*w        ( = P(a xor b = 1), the carry-propagate prob )
    t  = sp' + sq'        ( = (s_a + s_b)/2 )
    sr_{i+1} = u_i * sr_i + t_i             (sr = sign of carry, sr_0 = +1)
    res1_i = 0.5 - 2 * w_i * sr_i           (sr_i = carry-IN sign)
    res0_i = 0.5 + 2 * w_i * sr_i

The recurrence maps 1:1 onto the VectorEngine's tensor_tensor_scan
(state = data0*state + data1 along the free dim).  Layout is batch-major:
each SBUF partition holds R batch rows of 64 positions; rows are padded to 65
with a reset element (u=0, t=1) so one scan instruction chains all R rows and
re-initializes the carry to +1 at each row boundary.  The scan output shifted
right by one column is then exactly the per-position exclusive carry-in.

Sharding: pure data parallel, B=262144 split as 32768 rows per NeuronCore,
zero cross-core communication.
"""

import os
import sys

import numpy as np

for _p in ("/opt/trn_rl_repo", "/root/.axon_site/_ro/trn_rl_repo"):
    if _p not in sys.path and os.path.isdir(_p):
        sys.path.append(_p)

from concourse import bacc, bass, mybir, tile
from concourse.bass_utils import run_bass_kernel_spmd

N_CORES = 8
B = 262144
L = 64
K = 2
B_LOCAL = B // N_CORES  # 32768
P = 128                 # SBUF partitions
R = 32                  # batch rows per partition per tile (default)

F32 = mybir.dt.float32
ALU = mybir.AluOpType
ACT_COPY = mybir.ActivationFunctionType.Copy


def build_program(
    reps: int = 1,
    r: int = R,
    io_bufs: int = 2,
    scr_bufs: int = 2,
    u_on_act: bool = False,
    r_list: list | None = None,
) -> bass.Bass:
    # per-tile row counts (per partition); default uniform r
    if r_list is None:
        n_tiles = B_LOCAL // (P * r)
        assert n_tiles * P * r == B_LOCAL
        r_list = [r] * n_tiles
    assert sum(r_list) * P == B_LOCAL, (sum(r_list) * P, B_LOCAL)
    r_max = max(r_list)
    starts = [0]
    for rr in r_list:
        starts.append(starts[-1] + rr * P)
    nc = bacc.Bacc(
        "TRN2",
        target_bir_lowering=False,
        debug=False,
        enable_asserts=False,
        num_devices=N_CORES,
    )

    d_op1 = nc.dram_tensor("op1", [B_LOCAL, L, K], F32, kind="ExternalInput").ap()
    d_op2 = nc.dram_tensor("op2", [B_LOCAL, L, K], F32, kind="ExternalInput").ap()
    d_out = nc.dram_tensor("out", [B_LOCAL, L, K], F32, kind="ExternalOutput").ap()

    with tile.TileContext(nc) as tc:
        with (
            tc.tile_pool(name="io", bufs=io_bufs) as io_pool,
            tc.tile_pool(name="scr", bufs=scr_bufs) as scr_pool,
        ):
            n_tiles = len(r_list)
            for t in range(n_tiles * reps):
                t = t % n_tiles
                r = r_list[t]
                rows = slice(starts[t], starts[t + 1])

                in1 = io_pool.tile([P, r * L * K], F32, tag="in1")
                in2 = io_pool.tile([P, r * L * K], F32, tag="in2")
                nc.sync.dma_start(
                    out=in1[:],
                    in_=d_op1[rows].rearrange("(p r) l k -> p (r l k)", p=P),
                )
                nc.sync.dma_start(
                    out=in2[:],
                    in_=d_op2[rows].rearrange("(p r) l k -> p (r l k)", p=P),
                )

                # sp' = 0.5 - p, sq' = 0.5 - q  (strided read of the
                # interleaved [l, k] rows; ScalarE so the VectorEngine only
                # runs the 6 ops on the critical path)
                spp = scr_pool.tile([P, r * L], F32, tag="spp")
                sqp = scr_pool.tile([P, r * L], F32, tag="sqp")
                spp3 = spp[:].rearrange("p (r c) -> p r c", c=L)
                sqp3 = sqp[:].rearrange("p (r c) -> p r c", c=L)
                in1v = in1[:].rearrange("p (r l k) -> p r l k", l=L, k=K)
                in2v = in2[:].rearrange("p (r l k) -> p r l k", l=L, k=K)
                nc.scalar.activation(
                    out=spp3, in_=in1v[:, :, :, 1], func=ACT_COPY, bias=0.5, scale=-1.0
                )
                nc.scalar.activation(
                    out=sqp3, in_=in2v[:, :, :, 1], func=ACT_COPY, bias=0.5, scale=-1.0
                )

                # scan operands, rows padded to 65 with the reset element
                u_ext = scr_pool.tile([P, r * (L + 1)], F32, tag="u_ext")
                t_ext = scr_pool.tile([P, r * (L + 1)], F32, tag="t_ext")
                u3 = u_ext[:].rearrange("p (r c) -> p r c", c=L + 1)
                t3 = t_ext[:].rearrange("p (r c) -> p r c", c=L + 1)
                nc.gpsimd.memset(u3[:, :, L], 0.0)
                nc.gpsimd.memset(t3[:, :, L], 1.0)

                # t = sp' + sq'  (must read spp before the in-place w below)
                nc.vector.tensor_tensor(
                    out=t3[:, :, 0:L], in0=spp3, in1=sqp3, op=ALU.add
                )
                # w = sp' * sq'  (in place over spp)
                nc.vector.tensor_tensor(out=spp3, in0=spp3, in1=sqp3, op=ALU.mult)
                # u = -2*w + 0.5
                if u_on_act:
                    nc.scalar.activation(
                        out=u3[:, :, 0:L], in_=spp3, func=ACT_COPY,
                        bias=0.5, scale=-2.0,
                    )
                else:
                    nc.vector.tensor_scalar(
                        out=u3[:, :, 0:L],
                        in0=spp3,
                        scalar1=-2.0,
                        scalar2=0.5,
                        op0=ALU.mult,
                        op1=ALU.add,
                    )

                # carry scan: sr[n] = u[n]*sr[n-1] + t[n], written at offset 1
                # so srx (offset 0) is the exclusive carry-in; column 0 and the
                # reset columns hold +1 (carry=0).
                sr = scr_pool.tile([P, 1 + r * (L + 1)], F32, tag="sr")
                nc.gpsimd.memset(sr[:, 0:1], 1.0)
                nc.vector.tensor_tensor_scan(
                    out=sr[:, 1 : 1 + r * (L + 1)],
                    data0=u_ext[:],
                    data1=t_ext[:],
                    initial=1.0,
                    op0=ALU.mult,
                    op1=ALU.add,
                )
                srx = sr[:, 0 : r * (L + 1)].rearrange("p (r c) -> p r c", c=L + 1)[
                    :, :, 0:L
                ]

                # z = w * sr_in (in place over spp, which holds w)
                nc.vector.tensor_tensor(out=spp3, in0=spp3, in1=srx, op=ALU.mult)

                out_t = io_pool.tile([P, r * L * K], F32, tag="out")
                o4 = out_t[:].rearrange("p (r l k) -> p r l k", l=L, k=K)
                # res1 = -2*z + 0.5 (DVE), res0 = 2*z + 0.5 (ScalarE)
                nc.vector.tensor_scalar(
                    out=o4[:, :, :, 1],
                    in0=spp3,
                    scalar1=-2.0,
                    scalar2=0.5,
                    op0=ALU.mult,
                    op1=ALU.add,
                )
                nc.scalar.activation(
                    out=o4[:, :, :, 0], in_=spp3, func=ACT_COPY, bias=0.5, scale=2.0
                )

                nc.gpsimd.dma_start(
                    out=d_out[rows].rearrange("(p r) l k -> p (r l k)", p=P),
                    in_=out_t[:],
                )

    nc.compile()
    return nc


_NC = None


def _get_nc():
    global _NC
    if _NC is None:
        _NC = build_program()
    return _NC


def kernel(op1: np.ndarray, op2: np.ndarray) -> np.ndarray:
    op1 = np.ascontiguousarray(op1, dtype=np.float32)
    op2 = np.ascontiguousarray(op2, dtype=np.float32)
    assert op1.shape == (B, L, K) and op2.shape == (B, L, K)

    nc = _get_nc()
    in_maps = [
        {
            "op1": op1[i * B_LOCAL : (i + 1) * B_LOCAL],
            "op2": op2[i * B_LOCAL : (i + 1) * B_LOCAL],
        }
        for i in range(N_CORES)
    ]
    res = run_bass_kernel_spmd(nc, in_maps, core_ids=list(range(N_CORES)))
    return np.concatenate([res.results[i]["out"] for i in range(N_CORES)], axis=0)
